# revision 1
# baseline (speedup 1.0000x reference)
"""VisionZip text-aware token-selection kernel for Trainium2 (Bass/Tile).

Contract: kernel(**inputs) takes FULL inputs (B=32) and returns the FULL
output [32, 65, 1024]. Internally: pure data-parallel over 8 NeuronCores
(4 samples each).

Algorithm (per sample, all on device):
  score = 0.5*z(sum_h attn[h,0,1:]) + 0.5*z(cos(metric[1:], text))
  top-54 patches (+CLS) -> dominant mask m over 577 tokens (rank trick:
  rank_i = #{j: s_j > s_i} with s_0 = 1e30 sentinel; m = rank < 55)
  cums = cumsum(m) (upper-triangular ones matmul)
  pn = cums - i  (-position among remaining tokens)
  targets: remaining tokens with pn in {0,-52,...,-468}
  merge tokens: remaining non-targets; assigned to argmax_r <mn_i, Tn_r>
  Output rows = (C @ hidden) * recip where C[65, 577] is integer-valued:
    rows 0..54 : one-hot at the r-th selected token (ascending)
    rows 55+r  : cnt_r * onehot(tgt_r) + merge-membership one-hots
    recip rows 55+r = 1/cnt_r  (division applied on the PSUM->SBUF copy)
  hidden is passed as fp16 hi + fp16 lo (host split), so the big matmuls
  run as two fp16 passes (2x faster than fp32) with ~1e-6 total error;
  C's entries are integers <= 577, exact in fp16.
"""
import numpy as np

import sys
if '/opt/trn_rl_repo' not in sys.path:
    sys.path.insert(0, '/opt/trn_rl_repo')

import concourse.bacc as bacc
import concourse.tile as tile
from concourse import mybir
from concourse.bass_utils import run_bass_kernel_spmd

F32 = mybir.dt.float32
F16 = mybir.dt.float16
N_CORES = 8
BC = 4                      # samples per core
L = 577                     # tokens (incl CLS)
D = 1024
CK = 64
NH = 16
DOM = 54                    # dominant patches
NSEL = DOM + 1              # + CLS
CTX = 10
STEP = 52                   # (577-1-54) // 10
OUT_T = NSEL + CTX          # 65 output tokens
CHUNKS = [(0, 128), (128, 128), (256, 128), (384, 128), (512, 65)]
LPAD = 640
EQ = mybir.AluOpType


def _consts():
    c = {}
    c["c_ones1"] = np.ones((1, 128), np.float32)
    oh = np.zeros((BC * NH, BC), np.float32)
    for s in range(BC):
        oh[s * NH:(s + 1) * NH, s] = 1.0
    c["c_oh64"] = oh
    c["c_iden"] = np.eye(128, dtype=np.float32)
    c["c_ones128"] = np.ones((128, 128), np.float32)
    ut = (np.arange(128)[:, None] <= np.arange(128)[None, :]).astype(np.float32)
    c["c_ut128"] = ut
    c["c_onescol"] = np.ones((128, 1), np.float32)
    c["c_iota55"] = np.broadcast_to(
        (np.arange(NSEL) + 1.0).astype(np.float32), (128, 1, NSEL)).copy()
    iota52 = np.zeros((128, BC, CTX), np.float32)
    iota52[:, :, :] = -STEP * np.arange(CTX, dtype=np.float32)[None, None, :]
    c["c_iota52"] = iota52      # compare against pn = cums - i  (pn == -52r)
    ii = np.zeros((128, 5), np.float32)
    for ci, (off, _) in enumerate(CHUNKS):
        ii[:, ci] = off + np.arange(128)
    c["c_iotaI"] = ii
    selbc = np.zeros((BC, BC * 128), np.float32)
    for s in range(BC):
        selbc[s, s * 128:(s + 1) * 128] = 1.0
    c["c_selbc"] = selbc        # lhsT slice [BC,128] broadcasts row s to 128 parts
    sh = np.zeros((BC * CTX, OUT_T), np.float32)
    for s in range(BC):
        for r in range(CTX):
            sh[s * CTX + r, NSEL + r] = 1.0
    c["c_sh40"] = sh            # moves crec4[(s,r), s] -> partitions 55..64
    oh40 = np.zeros((BC * CTX, BC), np.float32)
    for s in range(BC):
        oh40[s * CTX:(s + 1) * CTX, s] = 1.0
    c["c_oh40"] = oh40
    return c


def build_nc(stage=99):
    nc = bacc.Bacc("TRN2", target_bir_lowering=False, debug=False)

    attn_d = nc.declare_dram_parameter("attn_row", [BC * NH, L], F32, isOutput=False)
    hhi_d = nc.declare_dram_parameter("h_hi", [BC, L, D], F16, isOutput=False)
    hlo_d = nc.declare_dram_parameter("h_lo", [BC, L, D], F16, isOutput=False)
    metric_d = nc.declare_dram_parameter("metric", [BC, L, CK], F32, isOutput=False)
    text_d = nc.declare_dram_parameter("text", [BC, CK], F32, isOutput=False)
    cshapes = {k: v.shape for k, v in _consts().items()}
    cdram = {k: nc.declare_dram_parameter(k, list(sh), F32, isOutput=False)
             for k, sh in cshapes.items()}
    out_d = nc.declare_dram_parameter("out", [BC, OUT_T, D], F32, isOutput=True)

    with tile.TileContext(nc) as tc:
        with (
            tc.tile_pool(name="persist", bufs=1) as pp,
            tc.tile_pool(name="hidpool", bufs=1) as hp,
            tc.tile_pool(name="scratch", bufs=2) as sp,
            tc.tile_pool(name="cpool", bufs=6) as cp,
            tc.tile_pool(name="ps_misc", bufs=3, space="PSUM") as ps_misc,
            tc.tile_pool(name="ps_bcast", bufs=2, space="PSUM") as ps_bcast,
            tc.tile_pool(name="ps_out", bufs=3, space="PSUM") as ps_out,
        ):
            pools = (pp, hp, sp, cp, ps_misc, ps_bcast, ps_out)
            _body(nc, stage, pools, attn_d, hhi_d, hlo_d, metric_d, text_d,
                  cdram, cshapes, out_d)
    nc.compile()
    return nc


def _body(nc, stage, pools, attn_d, hhi_d, hlo_d, metric_d, text_d,
          cdram, cshapes, out_d):
    pp, hp, sp, cp, ps_misc, ps_bcast, ps_out = pools
    V = nc.vector
    A = nc.scalar
    T = nc.tensor
    DMA = nc.sync

    def dump(n):
        d = sp.tile([BC, 512], F32, tag="dump")
        V.memset(d[:], float(n))
        DMA.dma_start(out_d[:, 0, 0:512], d[:])

    # ---- constant + small input DMAs ----
    csb = {}
    for k, sh in cshapes.items():
        t = pp.tile(list(sh), F32, tag=k)
        DMA.dma_start(t[:], cdram[k][:])
        csb[k] = t
    attn_sb = pp.tile([BC * NH, L], F32, tag="attn_sb")
    DMA.dma_start(attn_sb[:], attn_d[:])
    text_sb = pp.tile([BC, CK], F32, tag="text_sb")
    DMA.dma_start(text_sb[:], text_d[:])
    mt = []
    for ci, (off, k) in enumerate(CHUNKS):
        t = pp.tile([128, BC, CK], F32, tag=f"mt{ci}")
        DMA.dma_start(t[0:k, :, :],
                      metric_d[:, off:off + k, :].rearrange("s l c -> l s c"))
        mt.append(t)

    # ---- hidden hi/lo DMAs (big; stream in the background) ----
    hid = []      # hid[s][ci] -> (hi, lo) [128, 1024] f16 (chunk 4: 65 rows)
    for s in range(BC):
        row = []
        for ci, (off, k) in enumerate(CHUNKS):
            thi = hp.tile([128, D], F16, tag=f"hh{s}_{ci}")
            DMA.dma_start(thi[0:k, :], hhi_d[s, off:off + k, :])
            tlo = hp.tile([128, D], F16, tag=f"hl{s}_{ci}")
            DMA.dma_start(tlo[0:k, :], hlo_d[s, off:off + k, :])
            row.append((thi, tlo))
        hid.append(row)

    if stage <= 1:
        return dump(1)

    # ---- text_n ----
    tsc = sp.tile([BC, CK], F32, tag="tsc")
    tss = pp.tile([BC, 1], F32, tag="tss")
    V.tensor_mul(tsc[:], text_sb[:], text_sb[:])
    V.tensor_reduce(tss[:], tsc[:], axis=mybir.AxisListType.X, op=EQ.add)
    tst = pp.tile([BC, 1], F32, tag="tst")
    A.activation(tst[:], tss[:], mybir.ActivationFunctionType.Sqrt)
    trc = pp.tile([BC, 1], F32, tag="trc")
    V.reciprocal(trc[:], tst[:])
    textn = pp.tile([BC, CK], F32, tag="textn")
    V.tensor_scalar_mul(textn[:], text_sb[:], trc[:])

    # textb: [128, (s,c)] broadcast of text_n along partitions
    tb_ps = ps_misc.tile([128, BC * CK], F32, tag="ps")
    for s in range(BC):
        T.matmul(tb_ps[:, s * CK:(s + 1) * CK],
                 csb["c_selbc"][:, s * 128:(s + 1) * 128],
                 textn[:, :], start=True, stop=True)
    textb = pp.tile([128, BC, CK], F32, tag="textb")
    A.copy(textb[:].rearrange("p s c -> p (s c)"), tb_ps[:, :])

    # ---- Sd + cos in one [BC, 2, LPAD] tile (seg 0 = Sd, seg 1 = cos) ----
    sdcos = pp.tile([BC, 2, LPAD], F32, tag="sdcos")
    sd_ps1 = ps_bcast.tile([BC, 512], F32, tag="psb")
    sd_ps2 = ps_misc.tile([BC, L - 512], F32, tag="ps")
    T.matmul(sd_ps1[:, :], csb["c_oh64"][:, :], attn_sb[:, 0:512],
             start=True, stop=True)
    T.matmul(sd_ps2[:, :], csb["c_oh64"][:, :], attn_sb[:, 512:L],
             start=True, stop=True)
    A.copy(sdcos[:, 0, 0:512], sd_ps1[:, :])
    A.copy(sdcos[:, 0, 512:L], sd_ps2[:, :])

    # ---- metric norms, mn, cos, dot ----
    mn = []
    rnorm_all = pp.tile([128, 5, BC, 1], F32, tag="rnorm_all")
    cosc = pp.tile([128, 5, BC], F32, tag="cosc")
    for ci, (off, k) in enumerate(CHUNKS):
        sq = sp.tile([128, BC, CK], F32, tag="sq")
        V.tensor_mul(sq[0:k], mt[ci][0:k], mt[ci][0:k])
        ssq = sp.tile([128, BC], F32, tag="ssq")
        V.tensor_reduce(ssq[0:k], sq[0:k], axis=mybir.AxisListType.X, op=EQ.add)
        srt = sp.tile([128, BC], F32, tag="srt")
        A.activation(srt[0:k], ssq[0:k], mybir.ActivationFunctionType.Sqrt)
        V.reciprocal(rnorm_all[0:k, ci, :, 0], srt[0:k])
        mnc = pp.tile([128, BC, CK], F32, tag=f"mn{ci}")
        V.tensor_tensor(mnc[0:k], mt[ci][0:k],
                        rnorm_all[0:k, ci].broadcast_to([k, BC, CK]), op=EQ.mult)
        mn.append(mnc)
        # dot with text_n -> cos
        dq = sp.tile([128, BC, CK], F32, tag="dq")
        V.tensor_mul(dq[0:k], mt[ci][0:k], textb[0:k])
        dsum = sp.tile([128, BC], F32, tag="dsum")
        V.tensor_reduce(dsum[0:k], dq[0:k], axis=mybir.AxisListType.X, op=EQ.add)
        V.tensor_mul(cosc[0:k, ci, :], dsum[0:k], rnorm_all[0:k, ci, :, 0])

    # cos -> row layout (seg 1 of sdcos)
    for ci, (off, k) in enumerate(CHUNKS):
        cps = ps_misc.tile([BC, 128], F32, tag="ps")
        T.transpose(cps[:, 0:k], cosc[0:k, ci, :], csb["c_iden"][0:k, 0:k])
        A.copy(sdcos[:, 1, off:off + k], cps[:, 0:k])

    if stage <= 2:
        return dump(2)

    # ---- z-scores (both rows at once) -> score_row ----
    score_row = pp.tile([BC, LPAD], F32, tag="score_row")
    zsum = sp.tile([BC, 2], F32, tag="zsum")
    V.tensor_reduce(zsum[:], sdcos[:, :, 1:L], axis=mybir.AxisListType.X, op=EQ.add)
    zmean = sp.tile([BC, 2, 1], F32, tag="zmean")
    V.tensor_scalar_mul(zmean[:, :, 0], zsum[:], 1.0 / (L - 1))
    xm = pp.tile([BC, 2, L - 1], F32, tag="xm")
    V.tensor_tensor(xm[:], sdcos[:, :, 1:L],
                    zmean[:].broadcast_to([BC, 2, L - 1]), op=EQ.subtract)
    scr = sp.tile([BC, 2, L - 1], F32, tag="zscr")
    V.tensor_mul(scr[:], xm[:], xm[:])
    zssq = sp.tile([BC, 2], F32, tag="zssq")
    V.tensor_reduce(zssq[:], scr[:], axis=mybir.AxisListType.X, op=EQ.add)
    zstd = sp.tile([BC, 2], F32, tag="zstd")
    A.activation(zstd[:], zssq[:], mybir.ActivationFunctionType.Sqrt,
                 scale=1.0 / (L - 2))
    zden = sp.tile([BC, 2], F32, tag="zden")
    V.tensor_scalar_add(zden[:], zstd[:], 1e-6)
    zinv = sp.tile([BC, 2, 1], F32, tag="zinv")
    V.reciprocal(zinv[:, :, 0], zden[:])
    zinvh = sp.tile([BC, 2, 1], F32, tag="zinvh")
    V.tensor_scalar_mul(zinvh[:, :, 0], zinv[:, :, 0], 0.5)
    zt = sp.tile([BC, 2, L - 1], F32, tag="zt")
    V.tensor_tensor(zt[:], xm[:], zinvh[:].broadcast_to([BC, 2, L - 1]),
                    op=EQ.mult)
    V.tensor_tensor(score_row[:, 1:L], zt[:, 0, :], zt[:, 1, :], op=EQ.add)
    V.memset(score_row[:, 0:1], 1.0e30)

    if stage <= 3:
        return dump(3)

    # ---- scoreT ----
    scoreT = pp.tile([128, 5, BC], F32, tag="scoreT")
    for ci, (off, k) in enumerate(CHUNKS):
        sps = ps_misc.tile([128, BC], F32, tag="ps")
        T.transpose(sps[0:k, :], score_row[:, off:off + k],
                    csb["c_iden"][0:BC, 0:BC])
        A.copy(scoreT[0:k, ci, :], sps[0:k, :])

    # ---- rank ----
    rank = pp.tile([128, 5, BC], F32, tag="rank")
    nc.gpsimd.memset(rank[:].rearrange("p c s -> p (c s)"), 1.0e9)
    for s in range(BC):
        bc_ps1 = ps_bcast.tile([128, 512], F32, tag="psb")
        T.matmul(bc_ps1[:, :], csb["c_selbc"][:, s * 128:(s + 1) * 128],
                 score_row[:, 0:512], start=True, stop=True)
        bc_ps2 = ps_misc.tile([128, L - 512], F32, tag="ps")
        T.matmul(bc_ps2[:, :], csb["c_selbc"][:, s * 128:(s + 1) * 128],
                 score_row[:, 512:L], start=True, stop=True)
        bcs = sp.tile([128, LPAD], F32, tag="bcs")
        A.copy(bcs[:, 0:512], bc_ps1[:, :])
        A.copy(bcs[:, 512:L], bc_ps2[:, :])
        for ci, (off, k) in enumerate(CHUNKS):
            g = sp.tile([128, LPAD], F32, tag="g")
            V.tensor_scalar(g[0:k, 0:L], bcs[0:k, 0:L],
                            scoreT[0:k, ci, s:s + 1], 0.0,
                            op0=EQ.is_gt, op1=EQ.add,
                            accum_out=rank[0:k, ci, s:s + 1])

    if stage <= 4:
        return dump(4)

    # ---- m, cums, pn ----
    msk = pp.tile([128, 5, BC, 1], F32, tag="msk")
    V.tensor_scalar(msk[:].rearrange("p c s o -> p (c s o)"),
                    rank[:].rearrange("p c s -> p (c s)"),
                    float(NSEL), None, op0=EQ.is_lt)
    cums = pp.tile([128, 5, BC, 1], F32, tag="cums")
    nc.gpsimd.memset(cums[:].rearrange("p c s o -> p (c s o)"), 0.0)
    for cm in range(5):
        cps2 = ps_misc.tile([128, BC], F32, tag="ps")
        for ck in range(cm + 1):
            lhs = csb["c_ut128"] if ck == cm else csb["c_ones128"]
            kk = CHUNKS[ck][1]
            T.matmul(cps2[0:CHUNKS[cm][1], :], lhs[0:kk, 0:CHUNKS[cm][1]],
                     msk[0:kk, ck, :, 0], start=(ck == 0), stop=(ck == cm))
        A.copy(cums[0:CHUNKS[cm][1], cm, :, 0], cps2[0:CHUNKS[cm][1], :])
    pn = pp.tile([128, 5, BC, 1], F32, tag="pn")       # pn = cums - i
    for ci in range(5):
        V.tensor_scalar(pn[:, ci, :, 0], cums[:, ci, :, 0],
                        csb["c_iotaI"][:, ci:ci + 1], None, op0=EQ.subtract)
    notm = pp.tile([128, 5, BC, 1], F32, tag="notm")
    V.tensor_scalar(notm[:].rearrange("p c s o -> p (c s o)"),
                    msk[:].rearrange("p c s o -> p (c s o)"),
                    0.5, None, op0=EQ.is_lt)

    if stage <= 5:
        return dump(5)

    # ---- Itgt, is_mrg ----
    itgt = []
    ismrg = pp.tile([128, 5, BC, 1], F32, tag="ismrg")
    nc.gpsimd.memset(ismrg[:].rearrange("p c s o -> p (c s o)"), 0.0)
    for ci, (off, k) in enumerate(CHUNKS):
        it = pp.tile([128, BC, CTX], F32, tag=f"itgt{ci}")
        V.tensor_tensor(it[0:k], csb["c_iota52"][0:k],
                        pn[0:k, ci].broadcast_to([k, BC, CTX]), op=EQ.is_equal)
        V.tensor_tensor(it[0:k], it[0:k],
                        notm[0:k, ci].broadcast_to([k, BC, CTX]), op=EQ.mult)
        itgt.append(it)
        tany = sp.tile([128, BC], F32, tag="tany")
        V.tensor_reduce(tany[0:k], it[0:k], axis=mybir.AxisListType.X, op=EQ.add)
        e = sp.tile([128, BC], F32, tag="e_mrg")
        V.tensor_mul(e[0:k], notm[0:k, ci, :, 0], tany[0:k])
        V.tensor_sub(ismrg[0:k, ci, :, 0], notm[0:k, ci, :, 0], e[0:k])
    # chunk-4 rows 65.. (tokens i > 576) stay 0 from the memset above

    if stage <= 6:
        return dump(6)

    # ---- mnT (per sample) ----
    mnT = []
    for s in range(BC):
        t = pp.tile([CK, LPAD], F32, tag=f"mnT{s}")
        for ci, (off, k) in enumerate(CHUNKS):
            tps = ps_misc.tile([CK, 128], F32, tag="ps")
            T.transpose(tps[:, 0:k], mn[ci][0:k, s, :], csb["c_iden"][0:k, 0:k])
            A.copy(t[:, off:off + k], tps[:, 0:k])
        mnT.append(t)

    # ---- Tn ----
    tn_sb = pp.tile([CK, BC, CTX], F32, tag="tn_sb")
    for s in range(BC):
        tn_ps = ps_misc.tile([CK, CTX], F32, tag="ps")
        for ci, (off, k) in enumerate(CHUNKS):
            T.matmul(tn_ps[:, :], mn[ci][0:k, s, :], itgt[ci][0:k, s, :],
                     start=(ci == 0), stop=(ci == 4))
        A.copy(tn_sb[:, s, :], tn_ps[:, :])

    # ---- sim, rowmax, eq, eqM ----
    eqm = []
    for ci, (off, k) in enumerate(CHUNKS):
        sim_sb = sp.tile([128, BC, CTX], F32, tag="sim_sb")
        for s in range(BC):
            sim_ps = ps_misc.tile([128, CTX], F32, tag="ps")
            T.matmul(sim_ps[0:k, :], mnT[s][:, off:off + k],
                     tn_sb[:, s, :], start=True, stop=True)
            A.copy(sim_sb[0:k, s, :], sim_ps[0:k, :])
        rmx = sp.tile([128, BC, 1], F32, tag="rmx")
        V.tensor_reduce(rmx[0:k, :, 0], sim_sb[0:k], axis=mybir.AxisListType.X,
                        op=EQ.max)
        em = pp.tile([128, BC, CTX], F32, tag=f"eqm{ci}")
        V.tensor_tensor(em[0:k], sim_sb[0:k],
                        rmx[0:k].broadcast_to([k, BC, CTX]), op=EQ.is_ge)
        V.tensor_tensor(em[0:k], em[0:k],
                        ismrg[0:k, ci].broadcast_to([k, BC, CTX]), op=EQ.mult)
        eqm.append(em)

    if stage <= 7:
        return dump(7)

    # ---- counts (row + col), cmax, cntb, recip65 ----
    cnt_ps = ps_misc.tile([1, BC * CTX], F32, tag="ps")
    for ci, (off, k) in enumerate(CHUNKS):
        T.matmul(cnt_ps[:, :], csb["c_onescol"][0:k, :],
                 eqm[ci][0:k].rearrange("p s c -> p (s c)"),
                 start=(ci == 0), stop=(ci == 4))
    cmax_row = sp.tile([1, BC * CTX], F32, tag="cmax_row")
    V.tensor_scalar_max(cmax_row[:], cnt_ps[:, :], 1.0)
    cntb_ps = ps_misc.tile([128, BC * CTX], F32, tag="ps")
    T.matmul(cntb_ps[:, :], csb["c_ones1"][:, :], cmax_row[:, :],
             start=True, stop=True)
    cntb = pp.tile([128, BC, CTX], F32, tag="cntb")
    A.copy(cntb[:].rearrange("p s c -> p (s c)"), cntb_ps[:, :])
    # column variant for the reciprocal path
    cntc_ps = ps_misc.tile([BC * CTX, 1], F32, tag="ps")
    for ci, (off, k) in enumerate(CHUNKS):
        T.matmul(cntc_ps[:, :], eqm[ci][0:k].rearrange("p s c -> p (s c)"),
                 csb["c_onescol"][0:k, :], start=(ci == 0), stop=(ci == 4))
    cmax_col = sp.tile([BC * CTX, 1], F32, tag="cmax_col")
    V.tensor_scalar_max(cmax_col[:], cntc_ps[:, :], 1.0)
    crec_col = sp.tile([BC * CTX, 1], F32, tag="crec_col")
    V.reciprocal(crec_col[:], cmax_col[:])
    crec4 = sp.tile([BC * CTX, BC], F32, tag="crec4")
    V.tensor_tensor(crec4[:], csb["c_oh40"][:, :],
                    crec_col[:].broadcast_to([BC * CTX, BC]), op=EQ.mult)
    r65_ps = ps_misc.tile([OUT_T, BC], F32, tag="ps")
    T.matmul(r65_ps[:, :], csb["c_sh40"][:, :], crec4[:, :],
             start=True, stop=True)
    recip65 = pp.tile([OUT_T, BC], F32, tag="recip65")
    A.copy(recip65[:, :], r65_ps[:, :])
    V.memset(recip65[0:NSEL, :], 1.0)

    if stage <= 8:
        return dump(8)

    # ---- C build (fp16, integer entries) + big fp16 matmuls + out DMA ----
    cts = []
    for ci, (off, k) in enumerate(CHUNKS):
        ct = cp.tile([128, BC, 80], F16, tag="C")
        V.tensor_tensor(ct[0:k, :, 0:NSEL],
                        csb["c_iota55"][0:k].broadcast_to([k, BC, NSEL]),
                        cums[0:k, ci].broadcast_to([k, BC, NSEL]),
                        op=EQ.is_equal)
        V.tensor_tensor(ct[0:k, :, 0:NSEL], ct[0:k, :, 0:NSEL],
                        msk[0:k, ci].broadcast_to([k, BC, NSEL]), op=EQ.mult)
        wct = sp.tile([128, BC, CTX], F32, tag="wct")
        V.tensor_mul(wct[0:k], itgt[ci][0:k], cntb[0:k])
        V.tensor_add(ct[0:k, :, NSEL:OUT_T], wct[0:k], eqm[ci][0:k])
        cts.append(ct)
    for s in range(BC):
        for n2 in range(2):
            po = ps_out.tile([OUT_T, 512], F32, tag="po")
            for ci, (off, k) in enumerate(CHUNKS):
                T.matmul(po[:, :], cts[ci][0:k, s, 0:OUT_T],
                         hid[s][ci][0][0:k, n2 * 512:(n2 + 1) * 512],
                         start=(ci == 0), stop=False)
            for ci, (off, k) in enumerate(CHUNKS):
                T.matmul(po[:, :], cts[ci][0:k, s, 0:OUT_T],
                         hid[s][ci][1][0:k, n2 * 512:(n2 + 1) * 512],
                         start=False, stop=(ci == 4))
            ob = sp.tile([OUT_T, 512], F32, tag="ob")
            V.tensor_scalar_mul(ob[:, :], po[:, :], recip65[:, s:s + 1])
            DMA.dma_start(out_d[s, :, n2 * 512:(n2 + 1) * 512], ob[:, :])


_NC = None


def _get_nc():
    global _NC
    if _NC is None:
        _NC = build_nc()
    return _NC


def shard_inputs(attn_weights, hidden_states, metric, text_emb):
    """Host-side shard: slice the CLS attention row; split batch across cores;
    split hidden into fp16 hi + fp16 lo."""
    B = attn_weights.shape[0]
    per = B // N_CORES
    attn_row = np.ascontiguousarray(attn_weights[:, :, 0, :])   # [B, 16, 577]
    h32 = np.asarray(hidden_states, np.float32)
    h_hi = h32.astype(np.float16)
    h_lo = (h32 - h_hi.astype(np.float32)).astype(np.float16)
    consts = _consts()
    in_maps = []
    for c in range(N_CORES):
        sl = slice(c * per, (c + 1) * per)
        m = {
            "attn_row": np.ascontiguousarray(
                attn_row[sl].reshape(per * NH, L)).astype(np.float32),
            "h_hi": np.ascontiguousarray(h_hi[sl]),
            "h_lo": np.ascontiguousarray(h_lo[sl]),
            "metric": np.ascontiguousarray(metric[sl]).astype(np.float32),
            "text": np.ascontiguousarray(text_emb[sl]).astype(np.float32),
        }
        m.update(consts)
        in_maps.append(m)
    return in_maps


def kernel(attn_weights, hidden_states, metric, text_emb):
    nc = _get_nc()
    in_maps = shard_inputs(attn_weights, hidden_states, metric, text_emb)
    res = run_bass_kernel_spmd(nc, in_maps, core_ids=list(range(N_CORES)))
    out = np.concatenate([r["out"] for r in res.results], axis=0)
    return out.astype(np.float32)



# revision 8
# speedup vs baseline: 1.0811x; 1.0811x over previous
"""VisionZip text-aware token-selection kernel for Trainium2 (Bass/Tile), v2.

Contract: kernel(**inputs) takes FULL inputs (B=32) and returns the FULL
output [32, 65, 1024]. Pure data-parallel over 8 NeuronCores (4 samples each).

v2 changes vs baseline:
  - hidden passed as single bf16 copy (fp16 runs at 2 cycles/row on the PE;
    bf16 runs at 1) -> big matmul phase ~4x faster, hidden DMA halved.
  - 1/cnt folded into the C matrix (C_ctx = itgt + eqm/cnt), so the PSUM
    result is final: no per-output recip multiply, fewer count matmuls.
  - affine score trick: rank order of 0.5*z(Sd)+0.5*z(cos) equals rank order
    of a*Sd + b*cos with a=0.5/(std_sd+eps), b=0.5/(std_cos+eps) (per-sample
    constants cancel) -> no mean-subtraction passes over [*,576].
  - column-major score pipeline: attn CLS-row and metric are host-transposed
    to [token(128p), chunk, ...] so Sd is one X-reduce and all selection
    tensors stay in the [128, 5, BC] layout; sums/sumsqs via ones-matmuls.
  - rank compare+accum ops read the broadcast scores directly from PSUM and
    are split across Vector and GpSimd.
  - sim/Tn matmuls batched into single PSUM tiles per chunk; rmx/eqm read
    PSUM directly (no sim copies).
  - hidden DMAs on the scalar-engine HWDGE ring, small inputs on the sync
    ring (parallel input streams); outputs also on the scalar ring.
"""
import numpy as np

import sys
if '/opt/trn_rl_repo' not in sys.path:
    sys.path.insert(0, '/opt/trn_rl_repo')

import concourse.bacc as bacc
import concourse.tile as tile
from concourse import mybir
from concourse.bass_utils import run_bass_kernel_spmd

F32 = mybir.dt.float32
BF16 = mybir.dt.bfloat16
NPBF16 = mybir.dt.np(mybir.dt.bfloat16)
N_CORES = 8
BC = 4                      # samples per core
L = 577                     # tokens (incl CLS)
LPAD = 640
D = 1024
CK = 64
NH = 16
DOM = 54
NSEL = DOM + 1              # + CLS
CTX = 10
STEP = 52                   # (577-1-54) // 10
OUT_T = NSEL + CTX          # 65 output tokens
CHUNKS = [(0, 128), (128, 128), (256, 128), (384, 128), (512, 65)]
EQ = mybir.AluOpType
AF = mybir.ActivationFunctionType
AX = mybir.AxisListType


def _consts():
    c = {}
    c["c_iden"] = np.eye(128, dtype=np.float32)
    ut = (np.arange(128)[:, None] <= np.arange(128)[None, :]).astype(NPBF16)
    c["c_utb"] = ut
    c["c_onesb"] = np.ones((128, 128), NPBF16)
    c["c_onescol"] = np.ones((128, 1), np.float32)
    c["c_ones1"] = np.ones((1, 128), np.float32)
    selbc = np.zeros((BC, BC * 128), np.float32)
    for s in range(BC):
        selbc[s, s * 128:(s + 1) * 128] = 1.0
    c["c_selbc"] = selbc
    c["c_iota55"] = (np.arange(NSEL) + 1.0).astype(np.float32).reshape(1, 1, NSEL) \
        .repeat(128, 0).copy()
    c["c_iota52"] = (-STEP * np.arange(CTX, dtype=np.float32)).reshape(1, 1, CTX) \
        .repeat(128, 0).copy()
    ii = np.zeros((128, 5, 1), np.float32)
    for ci, (off, _) in enumerate(CHUNKS):
        ii[:, ci, 0] = off + np.arange(128)
    c["c_iotaI"] = ii
    return c


_CONST_DTYPES = {"c_utb": BF16, "c_onesb": BF16}


def build_nc(stage=99):
    nc = bacc.Bacc("TRN2", target_bir_lowering=False, debug=False)

    attnT_d = nc.declare_dram_parameter("attnT", [128, 5, BC * NH], F32, isOutput=False)
    metricT_d = nc.declare_dram_parameter("metricT", [128, 5, BC, CK], F32, isOutput=False)
    text_d = nc.declare_dram_parameter("text", [BC, CK], F32, isOutput=False)
    hid_d = nc.declare_dram_parameter("hidb", [BC, L, D], BF16, isOutput=False)
    cshapes = {k: v.shape for k, v in _consts().items()}
    cdram = {k: nc.declare_dram_parameter(k, list(sh), _CONST_DTYPES.get(k, F32),
                                          isOutput=False)
             for k, sh in cshapes.items()}
    out_d = nc.declare_dram_parameter("out", [BC, OUT_T, D], F32, isOutput=True)

    with tile.TileContext(nc) as tc:
        with (
            tc.tile_pool(name="persist", bufs=1) as pp,
            tc.tile_pool(name="hidpool", bufs=1) as hp,
            tc.tile_pool(name="scratch", bufs=2) as sp,
            tc.tile_pool(name="ps_misc", bufs=2, space="PSUM") as ps_misc,
            tc.tile_pool(name="ps_bcs", bufs=2, space="PSUM") as ps_bcs,
            tc.tile_pool(name="ps_out", bufs=2, space="PSUM") as ps_out,
        ):
            pools = (pp, hp, sp, ps_misc, ps_bcs, ps_out)
            _body(nc, stage, pools, attnT_d, metricT_d, text_d, hid_d,
                  cdram, cshapes, out_d)
    nc.compile()
    return nc


def _body(nc, stage, pools, attnT_d, metricT_d, text_d, hid_d,
          cdram, cshapes, out_d):
    pp, hp, sp, ps_misc, ps_bcs, ps_out = pools
    V = nc.vector
    A = nc.scalar
    G = nc.gpsimd
    T = nc.tensor
    DMA = nc.sync          # small inputs: sync-engine HWDGE ring
    DMA2 = nc.scalar       # hidden + outputs: scalar-engine HWDGE ring

    def dump(n):
        d = sp.tile([BC, 512], F32, tag="dump")
        V.memset(d[:], float(n))
        DMA.dma_start(out_d[:, 0, 0:512], d[:])

    # ---- input DMAs ----
    attnT = pp.tile([128, 5, BC * NH], F32, tag="attnT")
    DMA.dma_start(attnT[:], attnT_d[:])
    text_sb = pp.tile([BC, CK], F32, tag="text_sb")
    DMA.dma_start(text_sb[:], text_d[:])
    mt = pp.tile([128, 5, BC, CK], F32, tag="mt")
    DMA.dma_start(mt[:], metricT_d[:])
    csb = {}
    for k, sh in cshapes.items():
        t = pp.tile(list(sh), _CONST_DTYPES.get(k, F32), tag=k)
        DMA.dma_start(t[:], cdram[k][:])
        csb[k] = t
    # hidden (big): scalar ring, streams while selection math runs
    hid = []
    for s in range(BC):
        row = []
        for ci, (off, k) in enumerate(CHUNKS):
            th = hp.tile([128, D], BF16, tag=f"h{s}_{ci}")
            DMA2.dma_start(th[0:k, :], hid_d[s, off:off + k, :])
            row.append(th)
        hid.append(row)

    if stage <= 1:
        return dump(1)

    # ---- text_n and its partition-broadcast ----
    tsc = sp.tile([BC, CK], F32, tag="tsc")
    V.tensor_mul(tsc[:], text_sb[:], text_sb[:])
    tss = pp.tile([BC, 1], F32, tag="tss")
    V.tensor_reduce(tss[:], tsc[:], axis=AX.X, op=EQ.add)
    tst = pp.tile([BC, 1], F32, tag="tst")
    A.activation(tst[:], tss[:], AF.Sqrt)
    trc = pp.tile([BC, 1], F32, tag="trc")
    V.reciprocal(trc[:], tst[:])
    textn = pp.tile([BC, CK], F32, tag="textn")
    V.tensor_scalar_mul(textn[:], text_sb[:], trc[:])
    tb_ps = ps_misc.tile([128, BC * CK], F32, tag="ps")
    for s in range(BC):
        T.matmul(tb_ps[:, s * CK:(s + 1) * CK],
                 csb["c_selbc"][:, s * 128:(s + 1) * 128],
                 textn[:, :], start=True, stop=True)
    textb = pp.tile([128, BC, CK], F32, tag="textb")
    A.copy(textb[:].rearrange("p s c -> p (s c)"), tb_ps[:, :])

    # ---- X tile: [128, 5, (sd s0..3 | cos s0..3 | sd^2 | cos^2)] ----
    X = pp.tile([128, 5, 16], F32, tag="X")
    # Sd: sum CLS-attention over 16 heads -> X[:, :, 0:4]
    V.tensor_reduce(X[:, :, 0:4], attnT[:].rearrange("p c (s h) -> p c s h", h=NH),
                    axis=AX.X, op=EQ.add)

    # ---- metric norms, mn, cos ----
    mn = pp.tile([128, 5, BC, CK], F32, tag="mn")
    rnorm = pp.tile([128, 5, BC, 1], F32, tag="rnorm")
    for ci, (off, k) in enumerate(CHUNKS):
        sq = sp.tile([128, BC, CK], F32, tag="sq")
        V.tensor_mul(sq[0:k], mt[0:k, ci], mt[0:k, ci])
        ssq = sp.tile([128, BC], F32, tag="ssq")
        V.tensor_reduce(ssq[0:k], sq[0:k], axis=AX.X, op=EQ.add)
        srt = sp.tile([128, BC], F32, tag="srt")
        A.activation(srt[0:k], ssq[0:k], AF.Sqrt)
        V.reciprocal(rnorm[0:k, ci, :, 0], srt[0:k])
        V.tensor_tensor(mn[0:k, ci], mt[0:k, ci],
                        rnorm[0:k, ci].broadcast_to([k, BC, CK]), op=EQ.mult)
        dq = sp.tile([128, BC, CK], F32, tag="dq")
        V.tensor_mul(dq[0:k], mt[0:k, ci], textb[0:k])
        dsum = sp.tile([128, BC], F32, tag="dsum")
        V.tensor_reduce(dsum[0:k], dq[0:k], axis=AX.X, op=EQ.add)
        V.tensor_mul(X[0:k, ci, 4:8], dsum[0:k], rnorm[0:k, ci, :, 0])

    # CLS excluded from z-stats
    V.memset(X[0:1, 0, 0:8], 0.0)
    # squares
    V.tensor_mul(X[:, :, 8:16], X[:, :, 0:8], X[:, :, 0:8])

    # ---- per-sample sums via ones-matmuls: [1, 16] ----
    st_ps = ps_misc.tile([1, 16], F32, tag="ps")
    for ci, (off, k) in enumerate(CHUNKS):
        T.matmul(st_ps[:, :], csb["c_onescol"][0:k, 0:1], X[0:k, ci, :],
                 start=(ci == 0), stop=(ci == 4))
    sums = pp.tile([1, 16], F32, tag="sums")
    A.copy(sums[:, :], st_ps[:, :])
    # var = (sumsq - sum^2/576)/575 ; ab = 0.5/(sqrt(var)+1e-6)
    musq = sp.tile([1, 8], F32, tag="musq")
    V.tensor_mul(musq[:], sums[:, 0:8], sums[:, 0:8])
    V.tensor_scalar_mul(musq[:], musq[:], -1.0 / (L - 1))
    var_ = sp.tile([1, 8], F32, tag="var_")
    V.tensor_add(var_[:], sums[:, 8:16], musq[:])
    stdv = sp.tile([1, 8], F32, tag="stdv")
    A.activation(stdv[:], var_[:], AF.Sqrt, scale=1.0 / (L - 2))
    V.tensor_scalar_add(stdv[:], stdv[:], 1e-6)
    inv = sp.tile([1, 8], F32, tag="inv")
    V.reciprocal(inv[:], stdv[:])
    ab_row = pp.tile([1, 8], F32, tag="ab_row")
    V.tensor_scalar_mul(ab_row[:], inv[:], 0.5)
    # partition-broadcast of ab: [128, 1, 8] PSUM
    abP = ps_misc.tile([128, 1, 8], F32, tag="ps")
    T.matmul(abP[:, 0, :], csb["c_ones1"][:, :], ab_row[:, :], start=True, stop=True)

    # ---- score_col = a*sd + b*cos ; CLS sentinel ----
    sc_t = sp.tile([128, 5, BC], F32, tag="sc_t")
    V.tensor_tensor(sc_t[:], X[:, :, 0:4],
                    abP[:, :, 0:4].broadcast_to([128, 5, 4]), op=EQ.mult)
    sc_u = sp.tile([128, 5, BC], F32, tag="sc_u")
    V.tensor_tensor(sc_u[:], X[:, :, 4:8],
                    abP[:, :, 4:8].broadcast_to([128, 5, 4]), op=EQ.mult)
    score_col = pp.tile([128, 5, BC], F32, tag="score_col")
    V.tensor_add(score_col[:], sc_t[:], sc_u[:])
    V.memset(score_col[0:1, 0, :], 1.0e30)

    if stage <= 2:
        return dump(2)

    # ---- score_row [BC, 640] via 5 transposes ----
    score_row = pp.tile([BC, LPAD], F32, tag="score_row")
    for ci, (off, k) in enumerate(CHUNKS):
        srp = ps_misc.tile([BC, 128], F32, tag="ps")
        T.transpose(srp[:, 0:k], score_col[0:k, ci, :], csb["c_iden"][0:k, 0:k])
        A.copy(score_row[:, off:off + k], srp[:, 0:k])

    # ---- mnT (overlaps rank below; PE/ACT while V/G rank) ----
    mnT = []
    for s in range(BC):
        t = pp.tile([CK, LPAD], F32, tag=f"mnT{s}")
        for ci, (off, k) in enumerate(CHUNKS):
            tps = ps_misc.tile([CK, 128], F32, tag="ps")
            T.transpose(tps[:, 0:k], mn[0:k, ci, s, :], csb["c_iden"][0:k, 0:k])
            A.copy(t[:, off:off + k], tps[:, 0:k])
        mnT.append(t)

    # ---- rank: per-sample broadcast (PSUM) + compare-accum (V/G split) ----
    rank = pp.tile([128, 5, BC], F32, tag="rank")
    G.memset(rank[:].rearrange("p c s -> p (c s)"), 1.0e9)
    for s in range(BC):
        bc_ps = ps_bcs.tile([128, LPAD], F32, tag="bcs")
        T.matmul(bc_ps[:, 0:512], csb["c_selbc"][:, s * 128:(s + 1) * 128],
                 score_row[:, 0:512], start=True, stop=True)
        T.matmul(bc_ps[:, 512:LPAD], csb["c_selbc"][:, s * 128:(s + 1) * 128],
                 score_row[:, 512:LPAD], start=True, stop=True)
        for ci, (off, k) in enumerate(CHUNKS):
            g = sp.tile([128, LPAD], BF16, tag="g")
            V.tensor_scalar(g[0:k, 0:L], bc_ps[0:k, 0:L],
                            score_col[0:k, ci, s:s + 1], 0.0,
                            op0=EQ.is_gt, op1=EQ.add,
                            accum_out=rank[0:k, ci, s:s + 1])

    if stage <= 3:
        return dump(3)

    # ---- msk (f32 + bf16), notm, cums, pn ----
    msk_f = pp.tile([128, 5, BC, 1], F32, tag="msk_f")
    V.tensor_scalar(msk_f[:].rearrange("p c s o -> p (c s o)"),
                    rank[:].rearrange("p c s -> p (c s)"),
                    float(NSEL), None, op0=EQ.is_lt)
    msk_b = pp.tile([128, 5, BC, 1], BF16, tag="msk_b")
    G.tensor_scalar(msk_b[:].rearrange("p c s o -> p (c s o)"),
                    rank[:].rearrange("p c s -> p (c s)"),
                    float(NSEL), None, op0=EQ.is_lt)
    notm = pp.tile([128, 5, BC, 1], F32, tag="notm")
    G.tensor_scalar(notm[:].rearrange("p c s o -> p (c s o)"),
                    msk_f[:].rearrange("p c s o -> p (c s o)"),
                    0.5, None, op0=EQ.is_lt)
    cums = pp.tile([128, 5, BC, 1], F32, tag="cums")
    G.memset(cums[:].rearrange("p c s o -> p (c s o)"), 0.0)
    for cm in range(5):
        kcm = CHUNKS[cm][1]
        cps = ps_misc.tile([128, BC], F32, tag="ps")
        for ck in range(cm + 1):
            lhs = csb["c_utb"] if ck == cm else csb["c_onesb"]
            kk = CHUNKS[ck][1]
            T.matmul(cps[0:kcm, :], lhs[0:kk, 0:kcm], msk_b[0:kk, ck, :, 0],
                     start=(ck == 0), stop=(ck == cm))
        A.copy(cums[0:kcm, cm, :, 0], cps[0:kcm, :])
    pn = pp.tile([128, 5, BC, 1], F32, tag="pn")
    V.tensor_tensor(pn[:, :, :, 0], cums[:, :, :, 0],
                    csb["c_iotaI"][:].broadcast_to([128, 5, BC]), op=EQ.subtract)

    if stage <= 4:
        return dump(4)

    # ---- itgt, ismrg ----
    itgt = pp.tile([128, 5, BC, CTX], F32, tag="itgt")
    ismrg = pp.tile([128, 5, BC, 1], F32, tag="ismrg")
    G.memset(ismrg[:].rearrange("p c s o -> p (c s o)"), 0.0)
    for ci, (off, k) in enumerate(CHUNKS):
        V.tensor_tensor(itgt[0:k, ci], csb["c_iota52"][0:k].broadcast_to([k, BC, CTX]),
                        pn[0:k, ci].broadcast_to([k, BC, CTX]), op=EQ.is_equal)
        V.tensor_tensor(itgt[0:k, ci], itgt[0:k, ci],
                        notm[0:k, ci].broadcast_to([k, BC, CTX]), op=EQ.mult)
        tany = sp.tile([128, BC], F32, tag="tany")
        V.tensor_reduce(tany[0:k], itgt[0:k, ci], axis=AX.X, op=EQ.add)
        omt = sp.tile([128, BC], F32, tag="omt")
        G.tensor_scalar(omt[0:k], tany[0:k], -1.0, 1.0, op0=EQ.mult, op1=EQ.add)
        G.tensor_mul(ismrg[0:k, ci, :, 0], notm[0:k, ci, :, 0], omt[0:k])

    # ---- Tn: [CK, BC, CTX] (batched PSUM) ----
    tn_ps = ps_misc.tile([CK, BC, CTX], F32, tag="ps")
    for s in range(BC):
        for ci, (off, k) in enumerate(CHUNKS):
            T.matmul(tn_ps[:, s, :], mn[0:k, ci, s, :], itgt[0:k, ci, s, :],
                     start=(ci == 0), stop=(ci == 4))
    tn_sb = pp.tile([CK, BC, CTX], F32, tag="tn_sb")
    A.copy(tn_sb[:].rearrange("p s c -> p (s c)"),
           tn_ps[:].rearrange("p s c -> p (s c)"))

    if stage <= 5:
        return dump(5)

    # ---- sim (batched PSUM per chunk), rmx, eqm ----
    eqm = pp.tile([128, 5, BC, CTX], F32, tag="eqm")
    for ci, (off, k) in enumerate(CHUNKS):
        sim_ps = ps_misc.tile([128, BC, CTX], F32, tag="ps")
        for s in range(BC):
            T.matmul(sim_ps[0:k, s, :], mnT[s][:, off:off + k], tn_sb[:, s, :],
                     start=True, stop=True)
        rmx = sp.tile([128, BC, 1], F32, tag="rmx")
        V.tensor_reduce(rmx[0:k, :, 0], sim_ps[0:k], axis=AX.X, op=EQ.max)
        V.tensor_tensor(eqm[0:k, ci], sim_ps[0:k],
                        rmx[0:k].broadcast_to([k, BC, CTX]), op=EQ.is_ge)
        V.tensor_tensor(eqm[0:k, ci], eqm[0:k, ci],
                        ismrg[0:k, ci].broadcast_to([k, BC, CTX]), op=EQ.mult)

    if stage <= 6:
        return dump(6)

    # ---- counts -> 1/cnt, partition-broadcast ----
    cnt_ps = ps_misc.tile([BC * CTX, 1], F32, tag="ps")
    for ci, (off, k) in enumerate(CHUNKS):
        T.matmul(cnt_ps[:, :], eqm[0:k, ci].rearrange("p s c -> p (s c)"),
                 csb["c_onescol"][0:k, :], start=(ci == 0), stop=(ci == 4))
    cmax = sp.tile([BC * CTX, 1], F32, tag="cmax")
    V.tensor_scalar_max(cmax[:], cnt_ps[:, :], 1.0)
    crec = sp.tile([BC * CTX, 1], F32, tag="crec")
    V.reciprocal(crec[:], cmax[:])
    crT_ps = ps_misc.tile([1, BC * CTX], F32, tag="ps")
    T.transpose(crT_ps[:, :], crec[:, :], csb["c_iden"][0:BC * CTX, 0:BC * CTX])
    crec_row = sp.tile([1, BC * CTX], F32, tag="crec_row")
    A.copy(crec_row[:, :], crT_ps[:, :])
    crb_ps = ps_misc.tile([128, BC, CTX], F32, tag="ps")
    T.matmul(crb_ps[:].rearrange("p s c -> p (s c)"), csb["c_ones1"][:, :],
             crec_row[:, :], start=True, stop=True)
    crb = pp.tile([128, BC, CTX], F32, tag="crb")
    A.copy(crb[:].rearrange("p s c -> p (s c)"),
           crb_ps[:].rearrange("p s c -> p (s c)"))

    if stage <= 7:
        return dump(7)

    # ---- C build (bf16): rows 0..54 one-hots, rows 55.. itgt + eqm/cnt ----
    cts = pp.tile([128, 5, BC, 80], BF16, tag="cts")
    for ci, (off, k) in enumerate(CHUNKS):
        dom = sp.tile([128, BC, NSEL], F32, tag="dom")
        V.tensor_tensor(dom[0:k], csb["c_iota55"][0:k].broadcast_to([k, BC, NSEL]),
                        cums[0:k, ci].broadcast_to([k, BC, NSEL]), op=EQ.is_equal)
        V.tensor_tensor(cts[0:k, ci, :, 0:NSEL], dom[0:k],
                        msk_f[0:k, ci].broadcast_to([k, BC, NSEL]), op=EQ.mult)
        wct = sp.tile([128, BC, CTX], F32, tag="wct")
        G.tensor_mul(wct[0:k], eqm[0:k, ci], crb[0:k])
        G.tensor_add(cts[0:k, ci, :, NSEL:OUT_T], wct[0:k], itgt[0:k, ci])

    if stage <= 8:
        return dump(8)

    # ---- big matmuls (bf16) + copy + out DMA ----
    for s in range(BC):
        for n2 in range(2):
            po = ps_out.tile([OUT_T, 512], F32, tag="po")
            for ci, (off, k) in enumerate(CHUNKS):
                T.matmul(po[:, :], cts[0:k, ci, s, 0:OUT_T],
                         hid[s][ci][0:k, n2 * 512:(n2 + 1) * 512],
                         start=(ci == 0), stop=(ci == 4))
            ob = sp.tile([OUT_T, 512], F32, tag="ob", bufs=3)
            if (s * 2 + n2) % 2 == 0:
                A.copy(ob[:, :], po[:, :])
            else:
                V.tensor_scalar_add(ob[:, :], po[:, :], 0.0)
            DMA2.dma_start(out_d[s, :, n2 * 512:(n2 + 1) * 512], ob[:, :])


_NC = None


def _get_nc():
    global _NC
    if _NC is None:
        _NC = build_nc()
    return _NC


def shard_inputs(attn_weights, hidden_states, metric, text_emb):
    """Host-side shard: slice CLS attention row, transpose to token-major
    column layout, cast hidden to bf16, split batch across cores."""
    B = attn_weights.shape[0]
    per = B // N_CORES
    attn_row = np.ascontiguousarray(attn_weights[:, :, 0, :], dtype=np.float32)
    h_b = np.asarray(hidden_states, np.float32).astype(NPBF16)
    met = np.asarray(metric, np.float32)
    consts = _consts()
    in_maps = []
    for c in range(N_CORES):
        sl = slice(c * per, (c + 1) * per)
        # attnT: [4,16,577] -> [577,4,16] -> pad 640 -> [128, 5, 64]
        at = attn_row[sl].transpose(2, 0, 1)                   # [577, 4, 16]
        atp = np.zeros((LPAD, per, NH), np.float32)
        atp[:L] = at
        atT = np.ascontiguousarray(
            atp.reshape(5, 128, per * NH).transpose(1, 0, 2))  # [128, 5, 64]
        # metricT: [4,577,64] -> [577,4,64] -> pad 640 -> [128, 5, 4, 64]
        mtc = met[sl].transpose(1, 0, 2)                       # [577, 4, 64]
        mtp = np.zeros((LPAD, per, CK), np.float32)
        mtp[:L] = mtc
        mtT = np.ascontiguousarray(
            mtp.reshape(5, 128, per, CK).transpose(1, 0, 2, 3))
        m = {
            "attnT": atT,
            "metricT": mtT,
            "text": np.ascontiguousarray(text_emb[sl]).astype(np.float32),
            "hidb": np.ascontiguousarray(h_b[sl]),
        }
        m.update(consts)
        in_maps.append(m)
    return in_maps


def kernel(attn_weights, hidden_states, metric, text_emb):
    nc = _get_nc()
    in_maps = shard_inputs(attn_weights, hidden_states, metric, text_emb)
    res = run_bass_kernel_spmd(nc, in_maps, core_ids=list(range(N_CORES)))
    out = np.concatenate([r["out"] for r in res.results], axis=0)
    return out.astype(np.float32)
